# revision 1
# baseline (speedup 1.0000x reference)
"""Trainium2 Bass kernel for nn_ChromaEncoder (sparse Cantor-route attention
encoder). Self-contained: host sharding/prep + Bass/Tile SPMD program for 8
NeuronCores, run via concourse bass_utils.run_bass_kernel_spmd.

Sharding: 8 cores = 4 batches x 2 half-cores.
  core c: batch b=c//2, half h=c%2.
  - Tokens permuted into Cantor-coordinate sorted order (host-side).
  - Attention/QKV/Wo: head-parallel across the pair (8 heads each, all
    1024 tokens); Wo partials summed via pair ReduceScatter.
  - FFN/LN/residual/outputs: token-parallel (own 512 sorted tokens).
  - Per layer: pair AllGather of x (bf16, feature-major), pair
    ReduceScatter of Wo partial (bf16, feature-major token-half blocks).
Attention is banded in sorted space: global token tile g (128 tokens)
attends within a 384-wide aligned window starting at tile clamp(g-1,0,5).
Activations are feature-major [feat(part), tok(free)]; x kept fp32,
matmul operands bf16.
"""

import numpy as np
import ml_dtypes
from contextlib import ExitStack

import concourse.bass as bass
import concourse.bacc as bacc
import concourse.tile as tile
import concourse.mybir as mybir
from concourse.masks import make_identity

BF16 = mybir.dt.bfloat16
F32 = mybir.dt.float32
AX = mybir.AxisListType.X
AF = mybir.ActivationFunctionType
OP = mybir.AluOpType
nbf = ml_dtypes.bfloat16

N_CORES = 8
GROUPS = [[0, 1], [2, 3], [4, 5], [6, 7]]
T = 1024
D = 1024
L_FULL = 4
FF = 4096
LAT = 512
HD = 64
WIN = 256
NEG = -30000.0      # bf16-representable large negative for additive mask
EPS = 1e-5


# ---------------------------------------------------------------- host prep
def cantor_perm_routes():
    coords = np.empty(T)
    for i in range(T):
        x = i / (T - 1)
        x = min(max(x, 1e-6), 1 - 1e-6)
        v = 0.0
        f = 0.5
        for _ in range(8):
            x *= 3.0
            dd = int(x)
            x -= dd
            if dd == 2:
                v += f
            f *= 0.5
        coords[i] = v
    dist = np.abs(coords[:, None] - coords[None, :])
    routes = np.argsort(dist, axis=1, kind='stable')[:, :16]
    perm = np.argsort(coords, kind='stable')
    pos = np.empty(T, np.int64)
    pos[perm] = np.arange(T)
    return perm, pos, routes


def circular_encoding():
    j = np.arange(D // 2)
    freq = (j + 1) / (D / 2)
    ang = 2.0 * np.pi * np.arange(12)[:, None] * freq[None, :] / 12
    enc = np.zeros((12, D), np.float32)
    enc[:, 0::2] = np.cos(ang)
    enc[:, 1::2] = np.sin(ang)
    return enc


S0 = np.clip(np.arange(8) * 128 - 64, 0, 768)


def host_static():
    perm, pos, routes = cantor_perm_routes()
    masks = np.full((8, 128, WIN), NEG, np.float32)
    for p in range(T):
        g, j = p // 128, p % 128
        s = int(S0[g])
        for kp in pos[routes[perm[p]]]:
            w = kp - s
            assert 0 <= w < WIN, (g, j, kp, s)
            masks[g, j, w] = 0.0
    return perm, pos, routes, masks


def prep_in_maps(inputs, layers=L_FULL):
    perm, pos, routes, masks = host_static()
    f = {k: np.asarray(v, np.float32) for k, v in inputs.items()}
    enc = circular_encoding()
    weff = f['W_emb'] + enc
    scales = (1.0 / (np.sqrt(HD) * np.abs(f['temp']))).astype(np.float32)

    x0 = np.einsum('btc,cd->btd', f['chroma'][:, perm], weff) + f['b_emb']

    def bf(a):
        return np.ascontiguousarray(np.asarray(a, np.float32).astype(nbf))

    def pslab(w, nk):
        return bf(w.reshape(nk, 128, -1))

    in_maps = []
    for c in range(N_CORES):
        b, h = c // 2, c % 2
        hs = slice(h * 512, (h + 1) * 512)
        ts = slice(h * 512, (h + 1) * 512)
        m = {}
        m['x0f'] = np.ascontiguousarray(
            x0[b][ts].T.reshape(8, 128, 512).astype(np.float32))
        m['masks'] = bf(masks)
        m['wq'] = np.stack([pslab(f['Wq'][l][:, hs], 8) for l in range(layers)])
        m['wk'] = np.stack([pslab(f['Wk'][l][:, hs], 8) for l in range(layers)])
        m['wv'] = np.stack([pslab(f['Wv'][l][:, hs], 8) for l in range(layers)])
        m['wo'] = np.stack([pslab(f['Wo'][l][hs, :], 4) for l in range(layers)])
        # w1 split into 8 column-eighths: [8, 8, 128, 512]
        m['w1'] = np.stack([
            np.ascontiguousarray(
                bf(f['W1'][l]).reshape(8, 128, 8, 512).transpose(2, 0, 1, 3))
            for l in range(layers)])
        m['w2'] = np.stack([pslab(f['W2'][l], 32) for l in range(layers)])
        m['wmu'] = pslab(f['Wmu'], 8)
        m['wlv'] = pslab(f['Wlv'], 8)

        def rowsb(vecs):
            return np.ascontiguousarray(np.stack(vecs, axis=1).astype(np.float32))

        m['bqs'] = rowsb([f['bq'][l][hs].reshape(4, 128)[i] * scales[l]
                          for l in range(layers) for i in range(4)])
        m['bks'] = rowsb([f['bk'][l][hs].reshape(4, 128)[i]
                          for l in range(layers) for i in range(4)])
        m['bvs'] = rowsb([f['bv'][l][hs].reshape(4, 128)[i]
                          for l in range(layers) for i in range(4)])
        m['bos'] = rowsb([f['bo'][l].reshape(8, 128)[i]
                          for l in range(layers) for i in range(8)])
        m['b1s'] = rowsb([f['b1'][l].reshape(32, 128)[i]
                          for l in range(layers) for i in range(32)])
        m['b2s'] = rowsb([f['b2'][l].reshape(8, 128)[i]
                          for l in range(layers) for i in range(8)])
        m['lgs'] = rowsb([f['ln_g'][l].reshape(8, 128)[i]
                          for l in range(layers) for i in range(8)])
        m['lbs'] = rowsb([f['ln_b'][l].reshape(8, 128)[i]
                          for l in range(layers) for i in range(8)])
        m['bmus'] = rowsb([f['bmu'].reshape(4, 128)[i] for i in range(4)])
        m['blvs'] = rowsb([f['blv'].reshape(4, 128)[i] for i in range(4)])
        in_maps.append(m)
    return in_maps, perm, scales


def unshard(results, perm):
    B = 4
    mu = np.empty((B, T, LAT), np.float32)
    lv = np.empty((B, T, LAT), np.float32)
    for c in range(N_CORES):
        b, h = c // 2, c % 2
        toks = perm[h * 512:(h + 1) * 512]
        mu[b, toks] = results[c]['muf'].reshape(LAT, 512).T
        lv[b, toks] = results[c]['lvf'].reshape(LAT, 512).T
    return mu, lv


# ---------------------------------------------------------------- device
def build_nc(layers=L_FULL, scales=None, reps=1, collectives=True,
             no_attn=False, no_ffn=False):
    assert scales is not None
    nc = bacc.Bacc("TRN2", target_bir_lowering=False, debug=False)

    din = {}
    def dt_in(name, shape, dt=BF16):
        din[name] = nc.dram_tensor(name, shape, dt, kind="ExternalInput")

    dt_in('x0f', [8, 128, 512], F32)
    dt_in('masks', [8, 128, WIN], BF16)
    dt_in('wq', [layers, 8, 128, 512]); dt_in('wk', [layers, 8, 128, 512])
    dt_in('wv', [layers, 8, 128, 512]); dt_in('wo', [layers, 4, 128, 1024])
    dt_in('w1', [layers, 8, 8, 128, 512]); dt_in('w2', [layers, 32, 128, 1024])
    dt_in('wmu', [8, 128, 512]); dt_in('wlv', [8, 128, 512])
    for nm, w in (('bqs', 4), ('bks', 4), ('bvs', 4), ('bos', 8), ('b1s', 32),
                  ('b2s', 8), ('lgs', 8), ('lbs', 8)):
        dt_in(nm, [128, layers * w], F32)
    dt_in('bmus', [128, 4], F32); dt_in('blvs', [128, 4], F32)

    muf = nc.dram_tensor('muf', [4, 128, 512], F32, kind="ExternalOutput")
    lvf = nc.dram_tensor('lvf', [4, 128, 512], F32, kind="ExternalOutput")

    ag_in = [nc.dram_tensor(f'ag_in{l}', [8, 128, 512], BF16) for l in range(layers)]
    ag_out = [nc.dram_tensor(f'ag_out{l}', [2, 8, 128, 512], BF16) for l in range(layers)]
    rs_in = [nc.dram_tensor(f'rs_in{l}', [2, 8, 128, 512], BF16) for l in range(layers)]
    rs_out = [nc.dram_tensor(f'rs_out{l}', [8, 128, 512], BF16) for l in range(layers)]

    with tile.TileContext(nc) as tc:
        with ExitStack() as ctx:
            sb = ctx.enter_context(tc.tile_pool(name="sb", bufs=1))

            ident = sb.tile([128, 128], BF16, tag="ident")
            make_identity(nc, ident[:])
            ones_col = sb.tile([128, 1], BF16, tag="ones")
            nc.vector.memset(ones_col[:], 1.0)
            eps_t = sb.tile([128, 1], F32, tag="epst")
            nc.vector.memset(eps_t[:], EPS)
            mask_sb = sb.tile([128, 8 * WIN], BF16, tag="mask")
            for g in range(8):
                nc.sync.dma_start(mask_sb[:, g * WIN:(g + 1) * WIN],
                                  din['masks'][g])
            bias_sb = {}
            for nm in ('bqs', 'bks', 'bvs', 'bos', 'b1s', 'b2s', 'lgs', 'lbs',
                       'bmus', 'blvs'):
                t = sb.tile([128, din[nm].shape[1]], F32, tag=nm)
                nc.sync.dma_start(t[:], din[nm].ap())
                bias_sb[nm] = t

            env = dict(nc=nc, tc=tc, sb=sb, din=din,
                       bias=bias_sb, mask=mask_sb, ident=ident,
                       ones=ones_col, eps=eps_t, layers=layers,
                       collectives=collectives, no_attn=no_attn,
                       no_ffn=no_ffn)

            for rep in range(reps):
                x, xb = [], []
                for c in range(8):
                    t = sb.tile([128, 512], F32, tag=f"x{c}")
                    nc.sync.dma_start(t[:], din['x0f'][c])
                    tb = sb.tile([128, 512], BF16, tag=f"xb{c}")
                    nc.vector.tensor_copy(tb[:], t[:])
                    x.append(t)
                    xb.append(tb)
                for l in range(layers):
                    x, xb = _layer(env, x, xb, ag_in[l], ag_out[l],
                                   rs_in[l], rs_out[l], l, float(scales[l]))
                # output projections
                with tc.tile_pool(name="po", bufs=3, space="PSUM") as po:
                    for wname, bname, out_d in (('wmu', 'bmus', muf),
                                                ('wlv', 'blvs', lvf)):
                        wsl = []
                        for kc in range(8):
                            wt = sb.tile([128, 512], BF16, tag=f"wsl{kc}")
                            nc.sync.dma_start(wt[:], din[wname][kc])
                            wsl.append(wt)
                        for n in range(4):
                            ps = po.tile([128, 512], F32, tag="pj")
                            for kc in range(8):
                                nc.tensor.matmul(
                                    ps[:], wsl[kc][:, n * 128:(n + 1) * 128],
                                    xb[kc][:], start=(kc == 0), stop=(kc == 7))
                            ot = sb.tile([128, 512], F32, tag="outt", bufs=1)
                            nc.scalar.activation(ot[:], ps[:], AF.Identity,
                                                 bias=bias_sb[bname][:, n:n + 1])
                            nc.sync.dma_start(out_d[n], ot[:])
    nc.compile()
    return nc


def _layer(env, x, xb, ag_in, ag_out, rs_in, rs_out, l, scale):
    nc, tc, sb = env['nc'], env['tc'], env['sb']
    din, bias_sb, mask_sb = env['din'], env['bias'], env['mask']
    ident, ones_col = env['ident'], env['ones']

    # ---- AllGather x -> xf [8][128, 1024] bf16 (global sorted tokens)
    for c in range(8):
        nc.sync.dma_start(ag_in[c], xb[c][:])
    if env['collectives']:
        nc.gpsimd.collective_compute(
            "AllGather", OP.bypass, ins=[ag_in.ap().opt()],
            outs=[ag_out.ap().opt()], replica_groups=GROUPS)
    else:
        for c in range(8):
            for bk in range(2):
                nc.sync.dma_start(ag_out[bk, c], ag_in[c])
    xf = []
    for c in range(8):
        t = sb.tile([128, 1024], BF16, tag=f"xf{c}")
        for bk in range(2):
            nc.sync.dma_start(t[:, bk * 512:(bk + 1) * 512], ag_out[bk, c])
        xf.append(t)

    # ---- q, k, v projections
    with tc.tile_pool(name=f"pp{l}", bufs=4, space="PSUM") as pp:
        def proj_qk(wname, bname, out_scale):
            slabs = []
            for kc in range(8):
                wt = sb.tile([128, 512], BF16, tag=f"wsl{kc}",
                             name=f"{wname}s{kc}")
                nc.sync.dma_start(wt[:], din[wname][l, kc])
                slabs.append(wt)
            outs = []
            for fc in range(4):
                ot = sb.tile([128, 1024], BF16, tag=f"{wname}o{fc}",
                             name=f"{wname}o{fc}")
                for bk in range(2):
                    ps = pp.tile([128, 512], F32, tag="pj", name="ps_qk")
                    for kc in range(8):
                        nc.tensor.matmul(
                            ps[:], slabs[kc][:, fc * 128:(fc + 1) * 128],
                            xf[kc][:, bk * 512:(bk + 1) * 512],
                            start=(kc == 0), stop=(kc == 7))
                    nc.scalar.activation(
                        ot[:, bk * 512:(bk + 1) * 512], ps[:], AF.Identity,
                        bias=bias_sb[bname][:, l * 4 + fc:l * 4 + fc + 1],
                        scale=out_scale)
                outs.append(ot)
            return outs

        q = proj_qk('wq', 'bqs', scale)
        k = proj_qk('wk', 'bks', 1.0)

        vslabs = []
        for kc in range(8):
            wt = sb.tile([128, 512], BF16, tag=f"wsl{kc}", name=f"wvs{kc}")
            nc.sync.dma_start(wt[:], din['wv'][l, kc])
            vslabs.append(wt)
        v = []
        for tt in range(8):
            ps = pp.tile([128, 512], F32, tag="pj", name="ps_v")
            for kc in range(8):
                nc.tensor.matmul(ps[:], xf[kc][:, tt * 128:(tt + 1) * 128],
                                 vslabs[kc][:], start=(kc == 0), stop=(kc == 7))
            vt = sb.tile([128, 512], BF16, tag=f"v{tt}", name=f"v{tt}")
            nc.scalar.copy(vt[:], ps[:])
            v.append(vt)

    # ---- attention
    ao = [sb.tile([128, 1024], BF16, tag=f"ao{fc}", name=f"ao{fc}")
          for fc in range(4)]
    if env['no_attn']:
        for fc in range(4):
            nc.vector.memset(ao[fc][:], 0.0)
    else:
        with tc.tile_pool(name=f"pa{l}", bufs=2, space="PSUM") as pa:
            for g in range(8):
                s0 = int(S0[g])
                for fc in range(4):
                    pvt = pa.tile([128, 128], F32, tag="pv", name="pvt")
                    for so in (0, 64):
                        sp = pa.tile([128, WIN], F32, tag="att", name="sp")
                        nc.tensor.matmul(
                            sp[:], q[fc][so:so + 64, g * 128:(g + 1) * 128],
                            k[fc][so:so + 64, s0:s0 + WIN],
                            start=True, stop=True)
                        s1 = sb.tile([128, WIN], F32, tag="s1", bufs=2,
                                     name="s1")
                        nc.vector.tensor_tensor(
                            s1[:], sp[:], mask_sb[:, g * WIN:(g + 1) * WIN],
                            OP.add)
                        den = sb.tile([128, 1], F32, tag="den", bufs=4,
                                      name="den")
                        if l == 0:
                            nmax = sb.tile([128, 1], F32, tag="nmax", bufs=4,
                                           name="nmax")
                            nc.vector.reduce_max(nmax[:], s1[:], axis=AX,
                                                 negate=True)
                            nc.scalar.activation(s1[:], s1[:], AF.Exp,
                                                 bias=nmax[:],
                                                 accum_out=den[:])
                        else:
                            nc.scalar.activation(s1[:], s1[:], AF.Exp,
                                                 accum_out=den[:])
                        rden = sb.tile([128, 1], F32, tag="rden", bufs=4,
                                       name="rden")
                        nc.vector.reciprocal(rden[:], den[:])
                        probs = sb.tile([128, WIN], BF16, tag="probs",
                                        bufs=2, name="probs")
                        nc.vector.tensor_scalar(probs[:], s1[:], rden[:],
                                                None, OP.mult)
                        # split window into segments aligned to 128-token
                        # v tiles; transpose each segment to the partition
                        # offset matching its v tile so lhsT/rhs bases agree.
                        segs = []          # (win_col0, ncols, vtile, poff)
                        p = s0
                        while p < s0 + WIN:
                            ta, off = p // 128, p % 128
                            n = min(128 - off, s0 + WIN - p)
                            segs.append((p - s0, n, ta, off))
                            p += n
                        tp = pa.tile([128, 128 * len(segs)], BF16, tag="attT",
                                     name="tp")
                        for i, (c0, n, ta, off) in enumerate(segs):
                            nc.tensor.transpose(
                                tp[off:off + n, i * 128:(i + 1) * 128],
                                probs[:, c0:c0 + n], ident[:])
                        pT = sb.tile([128, 128 * 3], BF16, tag="pT", bufs=2,
                                     name="pT")
                        nc.scalar.copy(pT[:, :128 * len(segs)],
                                       tp[:])
                        fsl = slice(fc * 128 + so, fc * 128 + so + 64)
                        for i, (c0, n, ta, off) in enumerate(segs):
                            nc.tensor.matmul(
                                pvt[so:so + 64, :],
                                v[ta][off:off + n, fsl],
                                pT[off:off + n, i * 128:(i + 1) * 128],
                                start=(i == 0), stop=(i == len(segs) - 1))
                    nc.scalar.activation(
                        ao[fc][:, g * 128:(g + 1) * 128], pvt[:], AF.Identity,
                        bias=bias_sb['bvs'][:, l * 4 + fc:l * 4 + fc + 1])

    # ---- Wo partial -> rs_in blocks, ReduceScatter
    with tc.tile_pool(name=f"pw{l}", bufs=3, space="PSUM") as pw:
        woslabs = []
        for kc in range(4):
            wt = sb.tile([128, 1024], BF16, tag=f"wbig{kc}", name=f"wos{kc}")
            nc.sync.dma_start(wt[:], din['wo'][l, kc])
            woslabs.append(wt)
        for bk in range(2):
            for n in range(8):
                ps = pw.tile([128, 512], F32, tag="pj", name="ps_wo")
                for kc in range(4):
                    nc.tensor.matmul(
                        ps[:], woslabs[kc][:, n * 128:(n + 1) * 128],
                        ao[kc][:, bk * 512:(bk + 1) * 512],
                        start=(kc == 0), stop=(kc == 3))
                wot = sb.tile([128, 512], BF16, tag="wot", bufs=2,
                              name="wot")
                nc.scalar.copy(wot[:], ps[:])
                nc.sync.dma_start(rs_in[bk, n], wot[:])
    if env['collectives']:
        nc.gpsimd.collective_compute(
            "ReduceScatter", OP.add, ins=[rs_in.ap().opt()],
            outs=[rs_out.ap().opt()], replica_groups=GROUPS)
    else:
        for n in range(8):
            nc.sync.dma_start(rs_out[n], rs_in[0, n])

    # ---- residual 1 + LN1
    xr = []
    for c in range(8):
        at = sb.tile([128, 512], BF16, tag=f"a{c % 4}", name=f"a{c}")
        nc.sync.dma_start(at[:], rs_out[c])
        t = sb.tile([128, 512], F32, tag=f"xr{c}", name=f"xr{c}")
        nc.vector.scalar_tensor_tensor(
            t[:], at[:], bias_sb['bos'][:, l * 8 + c:l * 8 + c + 1], x[c][:],
            OP.add, OP.add)
        xr.append(t)
    x, xb = _layernorm(env, xr, l)

    if env['no_ffn']:
        xr = []
        for c in range(8):
            t = sb.tile([128, 512], F32, tag=f"xr{c}", name=f"xrf{c}")
            nc.vector.tensor_copy(t[:], x[c][:])
            xr.append(t)
        x, xb = _layernorm(env, xr, l)
        return x, xb

    # ---- FFN: FFN1 eighth-slabs (pool 3) + FFN2 half-column groups (pool 4)
    with tc.tile_pool(name=f"pf{l}", bufs=1, space="PSUM") as pf:
        h = []
        for e8 in range(8):
            w1e = []
            for kc in range(8):
                wt = sb.tile([128, 512], BF16, tag=f"w1e{kc}", bufs=2,
                             name=f"w1e{e8}_{kc}")
                nc.sync.dma_start(wt[:], din['w1'][l, e8, kc])
                w1e.append(wt)
            for n4 in range(4):
                n = e8 * 4 + n4
                ps = pf.tile([128, 512], F32, tag="f1", bufs=3, name="ps_f1")
                for kc in range(8):
                    nc.tensor.matmul(ps[:],
                                     w1e[kc][:, n4 * 128:(n4 + 1) * 128],
                                     xb[kc][:], start=(kc == 0),
                                     stop=(kc == 7))
                if n % 2 == 0:
                    ht = sb.tile([128, 1024], BF16, tag=f"h{(n // 2) % 8}",
                                 bufs=2, name=f"h{n // 2}")
                    h.append(ht)
                nc.scalar.activation(
                    h[n // 2][:, (n % 2) * 512:(n % 2 + 1) * 512], ps[:],
                    AF.Gelu, bias=bias_sb['b1s'][:, l * 32 + n:l * 32 + n + 1])

        xr = [None] * 8
        for grp in range(2):
            pss = [pf.tile([128, 512], F32, tag=f"f2_{i}", name=f"ps_f2_{i}")
                   for i in range(4)]
            for kc in range(32):
                wt = sb.tile([128, 512], BF16, tag=f"w2h{kc % 3}", bufs=2,
                             name=f"w2h{grp}_{kc}")
                nc.sync.dma_start(
                    wt[:], din['w2'][l, kc][:, grp * 512:(grp + 1) * 512])
                for n4 in range(4):
                    nc.tensor.matmul(
                        pss[n4][:], wt[:, n4 * 128:(n4 + 1) * 128],
                        h[kc // 2][:, (kc % 2) * 512:(kc % 2 + 1) * 512],
                        start=(kc == 0), stop=(kc == 31))
            for n4 in range(4):
                n = grp * 4 + n4
                t = sb.tile([128, 512], F32, tag=f"xr{n}", name=f"xr2_{n}")
                nc.vector.scalar_tensor_tensor(
                    t[:], pss[n4][:],
                    bias_sb['b2s'][:, l * 8 + n:l * 8 + n + 1],
                    x[n][:], OP.add, OP.add)
                xr[n] = t
    x, xb = _layernorm(env, xr, l)
    return x, xb


def _layernorm(env, xr, l):
    nc, sb, tc = env['nc'], env['sb'], env['tc']
    bias_sb, ones_col = env['bias'], env['ones']
    xrb, sq = [], []
    for c in range(8):
        tb = sb.tile([128, 512], BF16, tag=f"xrb{c % 4}", name=f"xrb{c}")
        nc.vector.tensor_copy(tb[:], xr[c][:])
        xrb.append(tb)
        st = sb.tile([128, 512], BF16, tag=f"sq{c % 4}", name=f"sq{c}")
        nc.vector.tensor_mul(st[:], tb[:], tb[:])
        sq.append(st)
    with tc.tile_pool(name=f"pl{l}", bufs=1, space="PSUM") as pl:
        sum_ps = pl.tile([1, 512], F32, tag="lnsum", name="sum_ps")
        for c in range(8):
            nc.tensor.matmul(sum_ps[:], ones_col[:], xrb[c][:],
                             start=(c == 0), stop=(c == 7))
        sq_ps = pl.tile([1, 512], F32, tag="lnsq", name="sq_ps")
        for c in range(8):
            nc.tensor.matmul(sq_ps[:], ones_col[:], sq[c][:], start=(c == 0),
                             stop=(c == 7))
        mean = sb.tile([1, 512], F32, tag="rA", name="mean")
        nc.vector.tensor_scalar(mean[:], sum_ps[:], 1.0 / D, None, OP.mult)
        ex2 = sb.tile([1, 512], F32, tag="rB", name="ex2")
        nc.vector.tensor_scalar(ex2[:], sq_ps[:], 1.0 / D, None, OP.mult)
    m2 = sb.tile([1, 512], F32, tag="rC", name="m2")
    nc.vector.tensor_mul(m2[:], mean[:], mean[:])
    nc.vector.tensor_sub(ex2[:], ex2[:], m2[:])        # ex2 <- var
    sdev = sb.tile([1, 512], F32, tag="rD", name="sdev")
    nc.scalar.activation(sdev[:], ex2[:], AF.Sqrt, bias=env['eps'][:1, :])
    rstd = sb.tile([1, 512], F32, tag="rE", name="rstd")
    nc.vector.reciprocal(rstd[:], sdev[:])
    negmr = sb.tile([1, 512], F32, tag="rF", name="negmr")
    nc.vector.scalar_tensor_tensor(negmr[:], mean[:], -1.0, rstd[:],
                                   OP.mult, OP.mult)
    rsB = sb.tile([128, 512], F32, tag="rsB", name="rsB")
    nc.gpsimd.partition_broadcast(rsB[:], rstd[:])
    cB = sb.tile([128, 512], F32, tag="cB", name="cB")
    nc.gpsimd.partition_broadcast(cB[:], negmr[:])
    xo, xbo = [], []
    for c in range(8):
        nc.vector.tensor_mul(xr[c][:], xr[c][:], rsB[:])
        nc.vector.tensor_add(xr[c][:], xr[c][:], cB[:])
        xt = sb.tile([128, 512], F32, tag=f"x{c}", name=f"xn{c}")
        nc.scalar.activation(xt[:], xr[c][:], AF.Identity,
                             bias=bias_sb['lbs'][:, l * 8 + c:l * 8 + c + 1],
                             scale=bias_sb['lgs'][:, l * 8 + c:l * 8 + c + 1])
        xbt = sb.tile([128, 512], BF16, tag=f"xb{c}", name=f"xbn{c}")
        nc.vector.tensor_copy(xbt[:], xt[:])
        xo.append(xt)
        xbo.append(xbt)
    return xo, xbo


# ---------------------------------------------------------------- entry point
def kernel(**inputs):
    """Takes FULL unsharded inputs (numpy arrays keyed as in setup_inputs()),
    returns (mu, lv) full outputs."""
    from concourse import bass_utils

    in_maps, perm, scales = prep_in_maps(inputs, layers=L_FULL)
    nc = build_nc(layers=L_FULL, scales=scales)
    res = bass_utils.run_bass_kernel_spmd(nc, in_maps, list(range(N_CORES)))
    mu, lv = unshard(res.results, perm)
    return mu, lv

